# revision 45
# baseline (speedup 1.0000x reference)
"""AdaptiveCausalAttention on 8 TRN2 NeuronCores (Bass/Tile).

Sharding: core c = 2*b + g handles batch b (of 4) and heads 8g..8g+7 (of 16).

Per-core pipeline (built as one Tile graph, SPMD across 8 cores):
  1. QKV projection (f32r matmuls): qT/kT in [d, t] layout (q reversed along
     t), v in [t, d] layout augmented with a ones column (for softmax sums).
  2. Masks: both the adaptive-span and triangle-wave masks depend only on
     rel = q - k, so each head's combined multiplicative mask is a 1-D
     vector m[rel].  It is computed on-chip (DVE + ACT Sin with explicit
     range reduction), written to DRAM, and materialized into the NB+1
     distinct 128x256 score-tile masks per head with an all-positive-stride
     sliding-window (Hankel) DMA - q is processed reversed to make the
     Toeplitz structure Hankel.
  3. Attention, transposed: scoresT[k, q] tiles (f32r), exp on ACT
     (scale=1/8 folded in) straight to bf16, multiplicative mask on DVE,
     PV matmul with ones-augmented V gives y^T and the softmax sums in one
     accumulation.  Normalization: DVE reciprocal + K=1 ones-matmul
     replicate + DVE multiply.
  4. Pairwise (same-batch) bf16 AllGather of y^T, then the output
     projection of this core's token half (bf16 matmuls, f32 out).

Host work is limited to sharding/layout prep, O(H) scalar parameter
transforms, and the O(H) span_loss scalar.
"""
import math

import numpy as np

import concourse.bass as bass
import concourse.tile as tile
from concourse import bacc, mybir
from concourse.bass_utils import run_bass_kernel_spmd

F32 = mybir.dt.float32
F32R = mybir.dt.float32r
BF16 = mybir.dt.bfloat16
AF = mybir.ActivationFunctionType
OP = mybir.AluOpType

B, T, C = 4, 1024, 1024
H = 16
D = C // H
BLOCK = 1024
R = 32.0
SPAN_REG = 1e-4
P_MIN, P_MAX = 2.0, 64.0
RATIO_MIN, RATIO_MAX = -0.25, 0.25

HL = 8            # heads per core
NCORE = 8
QS = 256          # q-super width
NQS = T // QS     # 4 q-supers
KB = 128          # k-block
CHUNK = 4         # k-blocks per psum/exp chunk

_BUILD_CACHE = {}


def _sigmoid(x):
    return 1.0 / (1.0 + np.exp(-x))


def _head_params(span_params, period_weight, ratio_weight):
    span = _sigmoid(np.float64(1.0) * np.asarray(span_params)) * BLOCK
    period = P_MIN + (P_MAX - P_MIN) * _sigmoid(np.asarray(period_weight, np.float64))
    ratio = RATIO_MIN + (RATIO_MAX - RATIO_MIN) * _sigmoid(
        np.asarray(ratio_weight, np.float64))
    amplitude = period / 4.0
    offset = period * ratio
    return span, period, ratio, amplitude, offset


def _span_loss(span_params, period_weight, ratio_weight):
    span, period, ratio, amplitude, offset = _head_params(
        span_params, period_weight, ratio_weight)
    base = 1.0 / period + 2.0 * ratio + 0.5
    loss_terms = np.where(base < 1.0, base, 1.0 + (0.5 + offset - amplitude))
    return np.float32(SPAN_REG * np.sum((span + R) * loss_terms) / H)


def _rev_free_ap(ap, start, count):
    """AP writing/reading ap[:, start+count-1 .. start] (free step -1)."""
    return bass.AP(tensor=ap.tensor, offset=ap.offset + start + count - 1,
                   ap=[[ap.ap[0][0], ap.ap[0][1]], [-1, count]])


def _build(NB, reach_cap=10 ** 9, debug=False):
    """Build the SPMD graph.  NB = number of 128-wide k-blocks that cover
    [q-reach, q]; identical on all cores (max over heads).  reach_cap is an
    integer upper bound on max_h(span_h + R), used to trim the dead leading
    columns of the widest-delta score tiles."""
    NB = min(max(NB, 1), 8)
    L = 128 * NB + 384           # lmx length per head (rel in [-255, REL_HI])
    CH = L // 16                 # per-partition chunk of the lmx vector
    NMT = NB + 1                 # distinct mask tiles per head

    nc = bacc.Bacc("TRN2", target_bir_lowering=False, debug=False,
                   num_devices=NCORE)

    xT = nc.dram_tensor("xT", [C, T], F32R, kind="ExternalInput").ap()
    wqkv = nc.dram_tensor("wqkv", [C, 3 * HL * D], F32R, kind="ExternalInput").ap()
    wproj = nc.dram_tensor("wproj", [C // 2, C], BF16, kind="ExternalInput").ap()
    mpar = nc.dram_tensor("mpar", [128, 8], F32, kind="ExternalInput").ap()
    relg = nc.dram_tensor("relg", [128, CH], F32, kind="ExternalInput").ap()
    out = nc.dram_tensor("out", [C, T], BF16, kind="ExternalOutput").ap()
    dbg = None
    if debug:
        dbg = {
            "dq": nc.dram_tensor("dq", [128, 4 * T], F32R, kind="ExternalOutput").ap(),
            "dk": nc.dram_tensor("dk", [128, 4 * T], F32R, kind="ExternalOutput").ap(),
            "dv": nc.dram_tensor("dv", [128, 8 * HL * 65], BF16, kind="ExternalOutput").ap(),
            "dmask": nc.dram_tensor("dmask", [128, HL * NMT * QS], BF16, kind="ExternalOutput").ap(),
            "dy": nc.dram_tensor("dy", [128, 4 * T], BF16, kind="ExternalOutput").ap(),
            "dlmx": nc.dram_tensor("dlmx", [HL, L], BF16, kind="ExternalOutput").ap(),
        }

    groups = [[0, 1], [2, 3], [4, 5], [6, 7]]

    with tile.TileContext(nc) as tc:
        with tc.tile_pool(name="dram", bufs=1, space="DRAM") as dpool:
            lmx = dpool.tile([HL, L], BF16)
            cc_in = dpool.tile([C, T], BF16, name="cc_in")
            cc_out = dpool.tile([C, T], BF16, name="cc_out")
            _build_body(nc, tc, NB, L, CH, NMT, xT, wqkv, wproj, mpar, relg,
                        out, lmx, cc_in, cc_out, groups, dbg,
                        reach_cap=reach_cap)
    nc.compile()
    return nc


def _build_body(nc, tc, NB, L, CH, NMT, xT, wqkv, wproj, mpar, relg, out,
                lmx, cc_in, cc_out, groups, dbg=None, reach_cap=10 ** 9):
    from contextlib import ExitStack
    stack = ExitStack()

    # ---------------- persistent SBUF tiles ----------------
    persist = stack.enter_context(tc.tile_pool(name="persist", bufs=1))
    qt = [persist.tile([128, T], F32R, tag=f"qt{i}", name=f"qt{i}") for i in range(4)]
    kt = [persist.tile([128, T], F32R, tag=f"kt{i}", name=f"kt{i}") for i in range(4)]
    v_sb = persist.tile([128, 8 * (HL * 65)], BF16, tag="v", name="v_sb")     # [t | kb, h, d+1]
    mask = [persist.tile([128, 128 * (NMT + 1)], BF16, tag=f"mask{h}", name=f"mask{h}") for h in range(HL)]
    y_sb = [persist.tile([128, T], BF16, tag=f"y{i}", name=f"y{i}") for i in range(4)]
    ones64 = persist.tile([1, 64], BF16, tag="ones64", name="ones64")
    nc.gpsimd.memset(ones64[:], 1.0)
    nc.gpsimd.memset(v_sb[:], 1.0)

    # ---------------- phase 0: mask vectors + mask tiles ----------------
    mg = stack.enter_context(tc.tile_pool(name="mgen", bufs=1))
    if True:
        relt = mg.tile([128, CH], F32)
        nc.sync.dma_start(relt[:], relg[:])
        par = mg.tile([128, 8], F32)
        nc.sync.dma_start(par[:], mpar[:])
        ms = mg.tile([128, CH], F32)
        # ms = clip(rel*(-1/R) + (span+R)/R, 0, 1) * (rel >= 0)
        nc.vector.tensor_scalar(ms[:], relt[:], -1.0 / R, par[:, 0:1],
                                OP.mult, OP.add)
        nc.vector.tensor_scalar(ms[:], ms[:], 0.0, 1.0, OP.max, OP.min)
        cge = mg.tile([128, CH], F32)
        nc.vector.tensor_scalar(cge[:], relt[:], 0.0, None, OP.is_ge)
        nc.vector.tensor_tensor(ms[:], ms[:], cge[:], OP.mult)
        # triangle wave, mod-free range reduction:
        #   uk = rel*k/period + S_k  (S_k integer, makes uk >= 0)
        #   d  = uk - int(uk), folded into [-1/2, 1/2] whatever the cast's
        #        rounding mode; then cos(2*pi*uk) = sin(pi/2 - 2*pi*|d|).
        half_pi = mg.tile([128, 1], F32)
        nc.gpsimd.memset(half_pi[:], math.pi / 2.0)
        u = mg.tile([128, CH], F32)
        nc.vector.tensor_scalar(u[:], relt[:], par[:, 1:2], None, OP.mult)
        wsum = mg.tile([128, CH], F32)
        tk = mg.tile([128, CH], F32)
        ik = mg.tile([128, CH], mybir.dt.int32)
        fk = mg.tile([128, CH], F32)
        for ki, k in enumerate((1, 3, 5)):
            nc.vector.tensor_scalar(tk[:], u[:], float(k), par[:, 4 + ki:5 + ki],
                                    OP.mult, OP.add)
            nc.vector.tensor_copy(ik[:], tk[:])
            nc.vector.tensor_copy(fk[:], ik[:])
            nc.vector.tensor_tensor(tk[:], tk[:], fk[:], OP.subtract)
            gk = mg.tile([128, CH], F32, tag="gk")
            nc.vector.tensor_scalar(gk[:], tk[:], 0.5, None, OP.is_gt)
            nc.vector.scalar_tensor_tensor(tk[:], gk[:], -1.0, tk[:],
                                           OP.mult, OP.add)
            wk = wsum if k == 1 else mg.tile([128, CH], F32, tag="wk")
            nc.scalar.activation(tk[:], tk[:], AF.Abs)
            nc.scalar.activation(wk[:], tk[:], AF.Sin, bias=half_pi[:],
                                 scale=-2.0 * math.pi)
            if k != 1:
                nc.vector.scalar_tensor_tensor(wsum[:], wk[:], 1.0 / k ** 2,
                                               wsum[:], OP.mult, OP.add)
        warm = mg.tile([1, 1], F32)
        nc.scalar.activation(warm[:], half_pi[0:1, :], AF.Exp, bias=0.0,
                             scale=0.0)   # preload the Exp table set
        mt = mg.tile([128, CH], F32)
        # mt = clip(wsum*amp2c + off5, 0, 1);  amp2c = c1*amplitude/2
        nc.vector.tensor_scalar(mt[:], wsum[:], par[:, 2:3], par[:, 3:4],
                                OP.mult, OP.add)
        nc.vector.tensor_scalar(mt[:], mt[:], 0.0, 1.0, OP.max, OP.min)
        mn = mg.tile([128, CH], F32)
        nc.vector.tensor_tensor(mn[:], ms[:], mt[:], OP.min)
        nc.vector.tensor_scalar(mn[:], mn[:], 1e-6, None, OP.is_gt)
        mprod = mg.tile([128, CH], BF16)
        nc.vector.tensor_tensor(ms[:], ms[:], mt[:], OP.mult)
        nc.vector.tensor_tensor(mprod[:], ms[:], mn[:], OP.mult)
        # scatter to DRAM: partition p -> lmx flat [p*CH : p*CH+CH]
        lbase = lmx[:]
        dst = bass.AP(tensor=lbase.tensor, offset=lbase.offset,
                      ap=[[CH, 128], [1, CH]])
        nc.sync.dma_start(dst, mprod[:])
    # pre-open the attention/proj SBUF pools so the bump allocator gives
    # them addresses disjoint from the big QKV staging pools (avoids
    # cross-pool reuse hazards)
    pep = stack.enter_context(tc.tile_pool(name="pexp", bufs=3))
    rcp = stack.enter_context(tc.tile_pool(name="rcp", bufs=4))
    oop = stack.enter_context(tc.tile_pool(name="oo", bufs=3))

    # ---------------- phase 1: QKV ----------------
    with tc.tile_pool(name="xt", bufs=1) as xp, \
         tc.tile_pool(name="wq", bufs=1) as wqp, \
         tc.tile_pool(name="psqkv", bufs=4, space="PSUM") as pq:
        xt_all = xp.tile([128, 8 * T], F32R)
        WS = 3 * HL * D   # 1536
        wq_all = wqp.tile([128, 8 * WS], F32R)
        for cc in range(8):
            nc.sync.dma_start(xt_all[:, cc * T:(cc + 1) * T],
                              xT[cc * 128:(cc + 1) * 128, :])
            nc.sync.dma_start(wq_all[:, cc * WS:cc * WS + 512],
                              wqkv[cc * 128:(cc + 1) * 128, 0:512])
        for cc in range(8):
            nc.sync.dma_start(wq_all[:, cc * WS + 512:cc * WS + 1024],
                              wqkv[cc * 128:(cc + 1) * 128, 512:1024])
        for cc in range(8):
            nc.sync.dma_start(wq_all[:, cc * WS + 1024:(cc + 1) * WS],
                              wqkv[cc * 128:(cc + 1) * 128, 1024:1536])
        # q (m 0..3, reversed) and k (m 4..7)
        for m in range(8):
            for th in range(2):
                ps = pq.tile([128, 512], F32)
                for cc in range(8):
                    nc.tensor.matmul(
                        ps[:],
                        wq_all[:, cc * WS + m * 128: cc * WS + (m + 1) * 128],
                        xt_all[:, cc * T + th * 512: cc * T + th * 512 + 512],
                        start=(cc == 0), stop=(cc == 7))
                if m < 4:      # q: reversed copy  u = 1023 - t
                    dst = _rev_free_ap(qt[m][:], (1 - th) * 512, 512)
                    nc.vector.tensor_copy(dst, ps[:])
                else:          # k: plain copy on ACT
                    nc.scalar.copy(kt[m - 4][:, th * 512:th * 512 + 512], ps[:])
        # v: [t, d] layout with ones column, bf16
        for tt in range(8):
            ps = pq.tile([128, 512], F32)
            for cc in range(8):
                nc.tensor.matmul(
                    ps[:],
                    xt_all[:, cc * T + tt * 128: cc * T + tt * 128 + 128],
                    wq_all[:, cc * WS + 1024: cc * WS + 1536],
                    start=(cc == 0), stop=(cc == 7))
            vb = v_sb[:]
            dst = bass.AP(tensor=vb.tensor,
                          offset=vb.offset + tt * (HL * 65),
                          ap=[[vb.ap[0][0], 128], [65, 8], [1, 64]])
            nc.vector.tensor_copy(dst, ps[:])

    # ---------------- mask tile pulls (after QKV input DMAs) ----------------
    # one wide Hankel window per head: mask[h][j, c] = lmx[h][j + c]; the
    # per-k-block [128, 256] mask tiles are overlapping strided views of it.
    lbase = lmx[:]
    for h in range(HL):
        src = bass.AP(tensor=lbase.tensor, offset=lbase.offset + h * L,
                      ap=[[1, 128], [1, 128 * (NMT + 1)]])
        nc.sync.dma_start(mask[h][:], src)

    # ---------------- wproj prefetch (overlaps attention) ----------------
    wpp = stack.enter_context(tc.tile_pool(name="wp", bufs=1))
    wp_all = wpp.tile([128, 4 * 1024], BF16, name="wp_all")
    for dc in range(4):
        nc.sync.dma_start(wp_all[:, dc * 1024:(dc + 1) * 1024],
                          wproj[dc * 128:(dc + 1) * 128, :])

    # ---------------- phase 2: attention ----------------
    with tc.tile_pool(name="pss", bufs=2, space="PSUM") as pss, \
         tc.tile_pool(name="psy", bufs=2, space="PSUM") as psy:
        for h in range(HL):
            hh, ho = h // 2, (h % 2) * 64
            for p in range(NQS):
                kb_min = max(0, 2 * p + 1 - NB)
                kbs = list(range(kb_min, 2 * p + 2))
                t_start = max(0, NB - 1 - 2 * p)
                y_ps = psy.tile([65, 2 * QS], F32)
                qs_sl = slice(QS * (NQS - 1 - p), QS * (NQS - p))
                p_tiles = []
                # leading columns of the widest-delta (first) tile whose
                # rel exceeds the span reach everywhere are skipped entirely
                c0 = max(0, 256 * p - 128 * kbs[0] + 128 - reach_cap)
                c0 = min(c0, QS - 32)
                for ci in range(0, len(kbs), CHUNK):
                    ckbs = kbs[ci:ci + CHUNK]
                    w = len(ckbs) * QS
                    if ckbs[-1] == 2 * p + 1:
                        w -= QS // 2       # diagonal tile: right half is future
                    cs = c0 if ci == 0 else 0
                    s_ps = pss.tile([128, CHUNK * QS], F32)
                    for j, kb in enumerate(ckbs):
                        jc = cs if j == 0 else 0
                        wt_ = min(QS, w - j * QS) - jc
                        qsl = slice(QS * (NQS - 1 - p) + jc,
                                    QS * (NQS - 1 - p) + jc + wt_)
                        nc.tensor.matmul(
                            s_ps[:, j * QS + jc:j * QS + jc + wt_],
                            kt[hh][ho:ho + 64, kb * 128:(kb + 1) * 128],
                            qt[hh][ho:ho + 64, qsl],
                            start=True, stop=True)
                    p_sb = pep.tile([128, CHUNK * QS], BF16)
                    nc.scalar.activation(p_sb[:, cs:w], s_ps[:, cs:w], AF.Exp,
                                         bias=0.0, scale=1.0 / math.sqrt(D))
                    mbase = mask[h][:]
                    for j in range(len(ckbs)):
                        jc = cs if j == 0 else 0
                        wt_ = min(QS, w - j * QS) - jc
                        mview = bass.AP(
                            tensor=mbase.tensor,
                            offset=mbase.offset + 128 * (t_start + ci + j) + jc,
                            ap=[[mbase.ap[0][0], 128], [1, wt_]])
                        nc.vector.tensor_tensor(
                            p_sb[:, j * QS + jc:j * QS + jc + wt_],
                            p_sb[:, j * QS + jc:j * QS + jc + wt_], mview,
                            OP.mult)
                    p_tiles.append((p_sb, ckbs, w, cs))
                for p_sb, ckbs, wchunk, cs in p_tiles:
                    for j, kb in enumerate(ckbs):
                        jc = cs if j == 0 else 0
                        wt_ = min(QS, wchunk - j * QS) - jc
                        nc.tensor.matmul(
                            y_ps[0:65, jc:jc + wt_],
                            v_sb[:, kb * (HL * 65) + h * 65: kb * (HL * 65) + h * 65 + 65],
                            p_sb[:, j * QS + jc:j * QS + jc + wt_],
                            start=(kb == kbs[0]), stop=(kb == kbs[-1]),
                            skip_group_check=True)
                # normalize: r = 1/sums; replicate via K=1 ones-matmul
                r_sb = rcp.tile([1, QS], BF16)
                with nc.allow_low_precision(reason="softmax 1/sum in bf16"):
                    nc.vector.reciprocal(r_sb[:], y_ps[64:65, 0:QS])
                nc.tensor.matmul(y_ps[0:64, QS:2 * QS], ones64[:], r_sb[:],
                                 start=True, stop=True)
                rep = rcp.tile([64, QS], F32, tag="rep")
                nc.scalar.copy(rep[:], y_ps[0:64, QS:2 * QS])
                nc.vector.tensor_tensor(
                    y_sb[hh][ho:ho + 64, qs_sl], y_ps[0:64, 0:QS], rep[:],
                    OP.mult)

    if dbg is not None:
        for i in range(4):
            nc.sync.dma_start(dbg["dq"][:, i * T:(i + 1) * T], qt[i][:])
            nc.sync.dma_start(dbg["dk"][:, i * T:(i + 1) * T], kt[i][:])
            nc.sync.dma_start(dbg["dy"][:, i * T:(i + 1) * T], y_sb[i][:])
        nc.sync.dma_start(dbg["dv"][:], v_sb[:])
        for h in range(HL):
            nc.sync.dma_start(dbg["dmask"][:, h * NMT * QS:(h + 1) * NMT * QS],
                              mask[h][:])
        nc.sync.dma_start(dbg["dlmx"][:], lmx[:])

    # ---------------- phase 3: partial projection + pairwise AllReduce ----
    # Each core projects with ONLY its own heads' rows of Wproj (sliced on
    # the host, so the graph is SPMD-static), giving a full [C, T] partial.
    # A single pairwise f32 AllReduce sums the two partials; both cores end
    # with the complete output.  No y exchange, half the proj matmuls.
    with tc.tile_pool(name="pso", bufs=4, space="PSUM") as pso:
        for ncc in range(8):
            for uh in range(2):
                ps = pso.tile([128, 512], F32)
                for dc in range(4):
                    nc.tensor.matmul(
                        ps[:],
                        wp_all[:, dc * 1024 + ncc * 128: dc * 1024 + ncc * 128 + 128],
                        y_sb[dc][:, uh * 512:(uh + 1) * 512],
                        start=(dc == 0), stop=(dc == 3))
                o_sb = oop.tile([128, 512], BF16)
                if (ncc + uh) % 2 == 0:
                    nc.scalar.copy(o_sb[:], ps[:])
                else:
                    nc.vector.tensor_copy(o_sb[:], ps[:])
                nc.sync.dma_start(
                    cc_in[ncc * 128:(ncc + 1) * 128, uh * 512:(uh + 1) * 512],
                    o_sb[:])
        # 4 row-chunk AllReduces so reduce + out-DMA pipeline with the
        # projection groups that feed them
        for r in range(4):
            rows = slice(256 * r, 256 * (r + 1))
            if len(groups[0]) == 1:
                nc.sync.dma_start(cc_out[rows, :], cc_in[rows, :])
            else:
                nc.gpsimd.collective_compute(
                    "AllReduce", OP.add, ins=[cc_in[rows, :].opt()],
                    outs=[cc_out[rows, :].opt()], replica_groups=groups)
            nc.sync.dma_start(out[rows, :], cc_out[rows, :])
    stack.close()


def _prep_core_inputs(c, x, Wqkv, Wproj, span, period, amplitude, offset,
                      NB, L, CH):
    b, g = c // 2, c % 2
    heads = range(8 * g, 8 * g + 8)
    c1 = 16.0 / math.pi ** 2
    REL_HI = 128 * (NB - 1) + 255

    xT = np.ascontiguousarray(x[b].T).astype(np.float32)
    cols = []
    for base in (0, C, 2 * C):
        for h in heads:
            cols.append(Wqkv[:, base + h * D: base + (h + 1) * D])
    wqkv = np.ascontiguousarray(np.concatenate(cols, axis=1)).astype(np.float32)
    import ml_dtypes
    wproj = np.ascontiguousarray(Wproj[512 * g: 512 * (g + 1), :]
                                 ).astype(ml_dtypes.bfloat16)

    mpar = np.zeros((128, 8), np.float32)
    relg = np.zeros((128, CH), np.float32)
    for hl, h in enumerate(heads):
        for ch in range(16):
            p = hl * 16 + ch
            mpar[p, 0] = (span[h] + R) / R
            mpar[p, 1] = 1.0 / period[h]
            mpar[p, 2] = c1 * amplitude[h] / 2.0
            mpar[p, 3] = 0.5 + offset[h]
            for ki, k in enumerate((1, 3, 5)):
                mpar[p, 4 + ki] = float(np.ceil(256.0 * k / period[h]))
            u = ch * CH + np.arange(CH)
            relg[p] = (REL_HI - u).astype(np.float32)
    return {"xT": xT, "wqkv": wqkv, "wproj": wproj, "mpar": mpar, "relg": relg}


def kernel(x, Wqkv, Wproj, span_params, period_weight, ratio_weight):
    x = np.asarray(x, np.float32)
    Wqkv = np.asarray(Wqkv, np.float32)
    Wproj = np.asarray(Wproj, np.float32)
    span_params = np.asarray(span_params, np.float32)
    period_weight = np.asarray(period_weight, np.float32)
    ratio_weight = np.asarray(ratio_weight, np.float32)

    span, period, ratio, amplitude, offset = _head_params(
        span_params, period_weight, ratio_weight)
    reach = span + R
    NB = min(max(int(np.ceil(reach.max() / 128)) + 1, 1), 8)
    L = 128 * NB + 384
    CH = L // 16

    reach_cap = int(np.ceil(reach.max()))
    key = (NB, reach_cap)
    if key not in _BUILD_CACHE:
        _BUILD_CACHE[key] = _build(NB, reach_cap=reach_cap)
    nc = _BUILD_CACHE[key]

    in_maps = [_prep_core_inputs(c, x, Wqkv, Wproj, span, period, amplitude,
                                 offset, NB, L, CH) for c in range(NCORE)]

    y = np.empty((B, T, C), np.float32)
    for attempt in range(3):
        res = run_bass_kernel_spmd(nc, in_maps, core_ids=list(range(NCORE)))
        for b in range(B):
            # un-reverse u -> t and transpose [c, u] -> [t, c]
            y[b] = res.results[2 * b]["out"][:, ::-1].T
        # the first execution after process/device startup occasionally
        # returns garbage (transient device state); re-dispatch is cheap
        if np.isfinite(y).all() and np.abs(y).max() < 1e6:
            break
    loss = _span_loss(span_params, period_weight, ratio_weight)
    return y, loss
